# revision 1
# baseline (speedup 1.0000x reference)
"""GQA self-attention kernel for Trainium2, sharded over 8 NeuronCores.

Problem: x[4, 2048, 1024], 16 heads / 4 KV groups / head_dim 64.
Sharding: batch (4) x head-half (2 KV groups each) = 8 cores.

Per-core dataflow (all "transposed world": features on partitions):
  xT[1024,2048] -> qT[512,2048], kT[128,2048], vT[128,2048] (PE matmuls, f32r)
  vT --PE transpose--> v_aug[seq,65] tiles (ones column appended for softmax sums)
  scores s[k,q] = kT_g^T(d,kpos) . qT_h(d,q)   (K=64 contraction, PE-array halves)
  p = exp(s/8)  (ACT engine, straight from PSUM)
  av[65,q] += v_aug^T p  (row 64 = softmax denominator)
  avT_norm = av[0:64] * recip(av[64])  (DVE; denominator row replicated via K=1 matmul)
  yT[e,q] = Wo_p^T . avT_norm  -> DRAM
Host: y[b] = (yT[2b] + yT[2b+1]).T + bo
"""

import sys
import numpy as np

sys.path.insert(0, "/opt/trn_rl_repo")

from contextlib import ExitStack

import concourse.bass as bass
import concourse.bacc as bacc
import concourse.mybir as mybir
from concourse import tile
from concourse.bass_utils import run_bass_kernel_spmd

F32 = mybir.dt.float32
F32R = mybir.dt.float32r

B, S, E = 4, 2048, 1024
NUM_HEADS, NUM_GROUPS, D = 16, 4, 64
CQ = 512          # q cols per core (8 heads)
CK = 128          # kv cols per core (2 groups)
ET = E // 128     # 8 embed K-tiles
SC = S // 512     # 4 seq chunks of 512
KT = S // 128     # 16 key tiles of 128
QT = CQ // 128    # 4 qT partition tiles
SCALE = 1.0 / np.sqrt(np.float32(D))

_NC_CACHE = {}


def build_nc():
    nc = bacc.Bacc(None, target_bir_lowering=False)

    xT = nc.dram_tensor("xT", [E, S], F32R, kind="ExternalInput")
    wq = nc.dram_tensor("wq", [E, CQ], F32R, kind="ExternalInput")
    wk = nc.dram_tensor("wk", [E, CK], F32R, kind="ExternalInput")
    wv = nc.dram_tensor("wv", [E, CK], F32R, kind="ExternalInput")
    wo = nc.dram_tensor("wo", [CQ, E], F32R, kind="ExternalInput")
    bqd = nc.dram_tensor("bqd", [128, QT], F32, kind="ExternalInput")
    bkd = nc.dram_tensor("bkd", [128, 1], F32, kind="ExternalInput")
    bvd = nc.dram_tensor("bvd", [128, 1], F32, kind="ExternalInput")
    identd = nc.dram_tensor("identd", [128, 128], F32R, kind="ExternalInput")
    onesd = nc.dram_tensor("onesd", [128, 64], F32R, kind="ExternalInput")
    yT = nc.dram_tensor("yT", [E, S], F32, kind="ExternalOutput")

    with tile.TileContext(nc) as tc, ExitStack() as ctx, \
            nc.allow_low_precision(reason="f32r is bit-identical to f32 here"):
        const = ctx.enter_context(tc.tile_pool(name="const", bufs=1))
        wpool = ctx.enter_context(tc.tile_pool(name="wpool", bufs=1))
        big = ctx.enter_context(tc.tile_pool(name="big", bufs=1))
        xpool = ctx.enter_context(tc.tile_pool(name="xpool", bufs=2))
        ppool = ctx.enter_context(tc.tile_pool(name="ppool", bufs=4))
        avpool = ctx.enter_context(tc.tile_pool(name="avpool", bufs=2))
        ypool = ctx.enter_context(tc.tile_pool(name="ypool", bufs=3))
        npool = ctx.enter_context(tc.tile_pool(name="npool", bufs=3))
        psA = ctx.enter_context(tc.tile_pool(name="psA", bufs=3, space="PSUM"))
        psAV = ctx.enter_context(tc.tile_pool(name="psAV", bufs=2, space="PSUM"))
        psY = ctx.enter_context(tc.tile_pool(name="psY", bufs=2, space="PSUM"))
        psM = ctx.enter_context(tc.tile_pool(name="psM", bufs=1, space="PSUM"))

        # ---- constants ----
        ident = const.tile([128, 128], F32R)
        nc.sync.dma_start(out=ident[:], in_=identd[:, :])
        ones_row = const.tile([1, 64], F32R)
        nc.sync.dma_start(out=ones_row[:], in_=onesd[0:1, :])

        # ---- weights + biases ----
        wq_sb = wpool.tile([128, ET, CQ], F32R)
        for et in range(ET):
            nc.sync.dma_start(out=wq_sb[:, et, :], in_=wq[et * 128:(et + 1) * 128, :])
        wk_sb = wpool.tile([128, ET, CK], F32R)
        wv_sb = wpool.tile([128, ET, CK], F32R)
        for et in range(ET):
            nc.sync.dma_start(out=wk_sb[:, et, :], in_=wk[et * 128:(et + 1) * 128, :])
            nc.sync.dma_start(out=wv_sb[:, et, :], in_=wv[et * 128:(et + 1) * 128, :])
        wo_sb = wpool.tile([128, QT, E], F32R)
        for t in range(QT):
            nc.sync.dma_start(out=wo_sb[:, t, :], in_=wo[t * 128:(t + 1) * 128, :])
        bq_sb = wpool.tile([128, QT], F32)
        nc.sync.dma_start(out=bq_sb[:], in_=bqd[:, :])
        bk_sb = wpool.tile([128, 1], F32)
        nc.sync.dma_start(out=bk_sb[:], in_=bkd[:, :])
        bv_sb = wpool.tile([128, 1], F32)
        nc.sync.dma_start(out=bv_sb[:], in_=bvd[:, :])

        # ---- persistent activations ----
        qT_sb = big.tile([128, QT, S], F32R)      # 32KB/partition
        kT_sb = big.tile([128, S], F32R)          # 8KB
        vT_sb = big.tile([128, S], F32R)          # 8KB
        vaug = big.tile([128, 2 * KT, 65], F32R)  # v natural + ones col, per (g, kt)
        for g in range(2):
            for kt in range(KT):
                nc.sync.dma_start(out=vaug[:, g * KT + kt, 64:65],
                                  in_=onesd[:, 0:1])

        ADD = mybir.AluOpType.add
        MUL = mybir.AluOpType.mult

        # ================= phase 1: projections =================
        for sc in range(SC):
            lo = sc * 512
            xt = xpool.tile([128, ET, 512], F32R, tag="xt", name=f"xt{sc}")
            for et in range(ET):
                nc.sync.dma_start(
                    out=xt[:, et, :], in_=xT[et * 128:(et + 1) * 128, lo:lo + 512])
            # qT
            for t in range(QT):
                pq = psA.tile([128, 512], F32, tag="mm", name=f"pq{sc}_{t}")
                for et in range(ET):
                    nc.tensor.matmul(
                        pq[:], wq_sb[:, et, t * 128:(t + 1) * 128],
                        xt[:, et, :], start=(et == 0), stop=(et == ET - 1))
                nc.vector.tensor_scalar(
                    out=qT_sb[:, t, lo:lo + 512], in0=pq[:],
                    scalar1=bq_sb[:, t:t + 1], scalar2=None, op0=ADD)
            # kT / vT
            pk = psA.tile([128, 512], F32, tag="mm", name=f"pk{sc}")
            for et in range(ET):
                nc.tensor.matmul(pk[:], wk_sb[:, et, :], xt[:, et, :],
                                 start=(et == 0), stop=(et == ET - 1))
            nc.vector.tensor_scalar(out=kT_sb[:, lo:lo + 512], in0=pk[:],
                                    scalar1=bk_sb[:, 0:1], scalar2=None, op0=ADD)
            pv = psA.tile([128, 512], F32, tag="mm", name=f"pv{sc}")
            for et in range(ET):
                nc.tensor.matmul(pv[:], wv_sb[:, et, :], xt[:, et, :],
                                 start=(et == 0), stop=(et == ET - 1))
            nc.vector.tensor_scalar(out=vT_sb[:, lo:lo + 512], in0=pv[:],
                                    scalar1=bv_sb[:, 0:1], scalar2=None, op0=ADD)
            # transpose vT chunk -> v natural tiles (with ones col kept intact)
            for ktl in range(4):
                kt = sc * 4 + ktl
                ptr = psM.tile([128, 128], F32R, tag="misc", name=f"ptr{kt}")
                nc.tensor.transpose(ptr[:], vT_sb[:, kt * 128:(kt + 1) * 128], ident[:])
                for g in range(2):
                    nc.vector.tensor_copy(
                        out=vaug[:, g * KT + kt, 0:64], in_=ptr[:, g * 64:(g + 1) * 64])

        # ================= phase 2: attention + out-proj =================
        for qc in range(SC):
            lo = qc * 512
            avT = avpool.tile([128, QT, 512], F32R, tag="avT", name=f"avT{qc}")
            for h in range(8):
                t, g = h % 4, h // 4
                ph = g * 64
                avp = psAV.tile([128, 512], F32, tag="av", name=f"avp{qc}_{h}")
                for kt in range(KT):
                    sp = psA.tile([128, 512], F32, tag="mm", name=f"sp{qc}_{h}_{kt}")
                    nc.tensor.matmul(
                        sp[:],
                        kT_sb[ph:ph + 64, kt * 128:(kt + 1) * 128],
                        qT_sb[ph:ph + 64, t, lo:lo + 512],
                        start=True, stop=True)
                    pe = ppool.tile([128, 512], F32R, tag="pexp", name=f"pe{qc}_{h}_{kt}")
                    nc.scalar.activation(
                        pe[:], sp[:], mybir.ActivationFunctionType.Exp,
                        scale=float(SCALE))
                    nc.tensor.matmul(
                        avp[0:65, :], vaug[:, g * KT + kt, :], pe[:],
                        start=(kt == 0), stop=(kt == KT - 1))
                # normalize: avT_norm = av[0:64] * (1 / av[64])
                linv = npool.tile([1, 512], F32R, tag="linv", name=f"linv{qc}_{h}")
                nc.vector.reciprocal(linv[:], avp[64:65, :])
                lrp = psM.tile([128, 512], F32, tag="misc", name=f"lrp{qc}_{h}")
                nc.tensor.matmul(lrp[0:64, :], ones_row[:], linv[:],
                                 start=True, stop=True)
                lrep = npool.tile([64, 512], F32, tag="lrep", name=f"lrep{qc}_{h}")
                nc.vector.tensor_copy(out=lrep[:], in_=lrp[0:64, :])
                nc.vector.tensor_tensor(
                    out=avT[ph:ph + 64, t, :], in0=avp[0:64, :], in1=lrep[:], op=MUL)
            # out projection for this q chunk
            for et in range(ET):
                yp = psY.tile([128, 512], F32, tag="y", name=f"yp{qc}_{et}")
                for t in range(QT):
                    nc.tensor.matmul(
                        yp[:], wo_sb[:, t, et * 128:(et + 1) * 128],
                        avT[:, t, :], start=(t == 0), stop=(t == QT - 1))
                ysb = ypool.tile([128, 512], F32, tag="ysb", name=f"ysb{qc}_{et}")
                nc.vector.tensor_copy(out=ysb[:], in_=yp[:])
                nc.sync.dma_start(out=yT[et * 128:(et + 1) * 128, lo:lo + 512],
                                  in_=ysb[:])
    nc.compile()
    return nc


def _shard_inputs(x, Wq, bq, Wk, bk, Wv, bv, Wo, bo):
    """Build the 8 per-core input maps."""
    x = np.asarray(x, dtype=np.float32)
    in_maps = []
    for c in range(8):
        b, H = c // 2, c % 2
        heads = [8 * H + t for t in range(4)] + [8 * H + t + 4 for t in range(4)]
        # qT tile t holds (local head t -> partitions 0-63, local head t+4 -> 64-127)
        order = []
        for t in range(4):
            order.extend(range(heads[t] * 64, heads[t] * 64 + 64))
            order.extend(range(heads[t + 4] * 64, heads[t + 4] * 64 + 64))
        order = np.asarray(order)
        wq_p = np.ascontiguousarray(np.asarray(Wq, np.float32)[:, order])
        bq_p = np.ascontiguousarray(
            np.asarray(bq, np.float32)[order].reshape(4, 128).T)
        wo_p = np.ascontiguousarray(np.asarray(Wo, np.float32)[order, :])
        wk_s = np.ascontiguousarray(np.asarray(Wk, np.float32)[:, H * 128:(H + 1) * 128])
        wv_s = np.ascontiguousarray(np.asarray(Wv, np.float32)[:, H * 128:(H + 1) * 128])
        bk_s = np.ascontiguousarray(np.asarray(bk, np.float32)[H * 128:(H + 1) * 128]
                                    .reshape(128, 1))
        bv_s = np.ascontiguousarray(np.asarray(bv, np.float32)[H * 128:(H + 1) * 128]
                                    .reshape(128, 1))
        xT_b = np.ascontiguousarray(x[b].T)
        in_maps.append({
            "xT": xT_b, "wq": wq_p, "wk": wk_s, "wv": wv_s, "wo": wo_p,
            "bqd": bq_p, "bkd": bk_s, "bvd": bv_s,
            "identd": np.eye(128, dtype=np.float32),
            "onesd": np.ones((128, 64), dtype=np.float32),
        })
    return in_maps


def kernel(x, Wq, bq, Wk, bk, Wv, bv, Wo, bo, _trace=False):
    if "nc" not in _NC_CACHE:
        _NC_CACHE["nc"] = build_nc()
    nc = _NC_CACHE["nc"]
    in_maps = _shard_inputs(x, Wq, bq, Wk, bk, Wv, bv, Wo, bo)
    res = run_bass_kernel_spmd(nc, in_maps, list(range(8)), trace=_trace)
    bo = np.asarray(bo, dtype=np.float32)
    out = np.empty((B, S, E), dtype=np.float32)
    for b in range(B):
        yT = res.results[2 * b]["yT"] + res.results[2 * b + 1]["yT"]
        out[b] = yT.T + bo
    if _trace:
        return out, res
    return out



# revision 3
# speedup vs baseline: 1.5104x; 1.5104x over previous
"""GQA self-attention kernel for Trainium2, sharded over 8 NeuronCores.

Problem: x[4, 2048, 1024], 16 heads / 4 KV groups / head_dim 64.
Sharding: batch (4) x head-half (2 KV groups each) = 8 cores.

v2 dataflow (all-bf16 PE path, transposed world: features on partitions):
  xT[1024,2048]b16 -> qT[512,2048], kT[128,2048], vT[128,2048]  (PE, bf16)
  vT --PE transpose--> vaug[seq,65] tiles (ones col appended -> softmax sums)
  scores s[k,q] = kT_g^T(d,kpos) . qT_h(d,q): K=64 contraction, the two KV
    groups run CONCURRENTLY in the upper/lower 64-row halves of the PE array
    (row tiling via base_partition-derived tile_position)
  exp: one ACT instruction per [128,2048] PSUM span (2 kt tiles x 2 heads)
    -> bf16 p tiles; amortizes the ~352-cycle ACT fixed cost
  av[65,q] += vaug^T p (row 64 = softmax denominator)
  avT_norm = av[0:64] * recip(av[64])  (DVE; denom row replicated via K=1 MM)
  yT[e,q] = Wo_p^T . avT_norm -> DRAM (f32)
The attention loop is software-pipelined; Q-projection (next chunk) and
out-projection (previous chunk) matmuls are spliced into the PE queue as
fillers during exp waits so the PE stays dense (HAM stays warm).
Host: y[b] = (yT[2b] + yT[2b+1]).T + bo
"""

import sys
import numpy as np

sys.path.insert(0, "/opt/trn_rl_repo")

from collections import deque
from contextlib import ExitStack

import ml_dtypes

import concourse.bass as bass
import concourse.bacc as bacc
import concourse.mybir as mybir
from concourse import tile
from concourse.bass_utils import run_bass_kernel_spmd

F32 = mybir.dt.float32
BF16 = mybir.dt.bfloat16
NPBF16 = ml_dtypes.bfloat16

B, S, E = 4, 2048, 1024
NUM_HEADS, NUM_GROUPS, D = 16, 4, 64
CQ = 512          # q cols per core (8 heads)
CK = 128          # kv cols per core (2 groups)
ET = E // 128     # 8 embed K-tiles
SC = S // 512     # 4 seq chunks of 512
KT = S // 128     # 16 key tiles of 128
QT = CQ // 128    # 4 qT partition tiles (pair p: head p @0-63, head p+4 @64-127)
SCALE = 1.0 / np.sqrt(np.float32(D))

_NC_CACHE = {}


def build_nc():
    nc = bacc.Bacc(None, target_bir_lowering=False)

    xT = nc.dram_tensor("xT", [E, S], BF16, kind="ExternalInput")
    wq = nc.dram_tensor("wq", [E, CQ], BF16, kind="ExternalInput")
    wk = nc.dram_tensor("wk", [E, CK], BF16, kind="ExternalInput")
    wv = nc.dram_tensor("wv", [E, CK], BF16, kind="ExternalInput")
    wo = nc.dram_tensor("wo", [CQ, E], BF16, kind="ExternalInput")
    bqd = nc.dram_tensor("bqd", [128, QT], F32, kind="ExternalInput")
    bkd = nc.dram_tensor("bkd", [128, 1], F32, kind="ExternalInput")
    bvd = nc.dram_tensor("bvd", [128, 1], F32, kind="ExternalInput")
    identd = nc.dram_tensor("identd", [128, 128], BF16, kind="ExternalInput")
    onesd = nc.dram_tensor("onesd", [128, 64], BF16, kind="ExternalInput")
    yT = nc.dram_tensor("yT", [E, S], F32, kind="ExternalOutput")

    ADD = mybir.AluOpType.add
    MUL = mybir.AluOpType.mult
    EXP = mybir.ActivationFunctionType.Exp

    with tile.TileContext(nc) as tc, ExitStack() as ctx, \
            nc.allow_low_precision(reason="bf16 matmuls within 2e-2 tolerance"):
        const = ctx.enter_context(tc.tile_pool(name="const", bufs=1))
        wpool = ctx.enter_context(tc.tile_pool(name="wpool", bufs=1))
        big = ctx.enter_context(tc.tile_pool(name="big", bufs=1))
        pepool = ctx.enter_context(tc.tile_pool(name="pepool", bufs=4))
        avtpool = ctx.enter_context(tc.tile_pool(name="avtpool", bufs=2))
        npool = ctx.enter_context(tc.tile_pool(name="npool", bufs=2))
        ypool = ctx.enter_context(tc.tile_pool(name="ypool", bufs=2))
        # PSUM budget (16KB/partition = 8 banks, exact fit):
        #   s    [128,2048] f32  4 banks  (scores: 2 kt x 2 heads per j)
        #   avA  [128, 512] f32  1 bank   (head A attention accumulator)
        #   avB  [128, 512] f32  1 bank
        #   y    [128, 512] f32  1 bank   (out-proj + phase1 rotation)
        #   misc [128, 512] f32  1 bank   (q-proj fillers, lrp, transposes)
        ps = ctx.enter_context(tc.tile_pool(name="ps", bufs=1, space="PSUM"))

        # ---- constants ----
        ident = const.tile([128, 128], BF16)
        nc.sync.dma_start(out=ident[:], in_=identd[:, :])
        ones_row = const.tile([1, 64], BF16)
        nc.sync.dma_start(out=ones_row[:], in_=onesd[0:1, :])

        # ---- weights + biases (kv first: phase 1 needs them immediately) ----
        wk_sb = wpool.tile([128, ET, CK], BF16)
        wv_sb = wpool.tile([128, ET, CK], BF16)
        for et in range(ET):
            nc.sync.dma_start(out=wk_sb[:, et, :], in_=wk[et * 128:(et + 1) * 128, :])
            nc.sync.dma_start(out=wv_sb[:, et, :], in_=wv[et * 128:(et + 1) * 128, :])
        bk_sb = wpool.tile([128, 1], F32)
        nc.sync.dma_start(out=bk_sb[:], in_=bkd[:, :])
        bv_sb = wpool.tile([128, 1], F32)
        nc.sync.dma_start(out=bv_sb[:], in_=bvd[:, :])
        wq_sb = wpool.tile([128, ET, CQ], BF16)
        for et in range(ET):
            nc.sync.dma_start(out=wq_sb[:, et, :], in_=wq[et * 128:(et + 1) * 128, :])
        bq_sb = wpool.tile([128, QT], F32)
        nc.sync.dma_start(out=bq_sb[:], in_=bqd[:, :])
        wo_sb = wpool.tile([128, QT, E], BF16)
        for t in range(QT):
            nc.sync.dma_start(out=wo_sb[:, t, :], in_=wo[t * 128:(t + 1) * 128, :])

        # ---- persistent activations ----
        xT_sb = big.tile([128, ET, S], BF16)      # 32KB/partition
        for sc in range(SC):
            lo = sc * 512
            for et in range(ET):
                nc.sync.dma_start(
                    out=xT_sb[:, et, lo:lo + 512],
                    in_=xT[et * 128:(et + 1) * 128, lo:lo + 512])
        qT_sb = big.tile([128, QT, S], BF16)      # 16KB
        kT_sb = big.tile([128, S], BF16)          # 4KB
        vT_sb = big.tile([128, S], BF16)          # 4KB
        vaug = big.tile([128, 2 * KT, 65], BF16)  # v natural + ones col
        for g in range(2):
            for kt in range(KT):
                nc.sync.dma_start(out=vaug[:, g * KT + kt, 64:65],
                                  in_=onesd[:, 0:1])

        # ================= phase 1: K/V projections + transposes =================
        for sc in range(SC):
            lo = sc * 512
            pk = ps.tile([128, 512], F32, tag="avA", name=f"pk{sc}")
            for et in range(ET):
                nc.tensor.matmul(pk[:], wk_sb[:, et, :], xT_sb[:, et, lo:lo + 512],
                                 start=(et == 0), stop=(et == ET - 1))
            nc.vector.tensor_scalar(out=kT_sb[:, lo:lo + 512], in0=pk[:],
                                    scalar1=bk_sb[:, 0:1], scalar2=None, op0=ADD)
            pv = ps.tile([128, 512], F32, tag="avB", name=f"pv{sc}")
            for et in range(ET):
                nc.tensor.matmul(pv[:], wv_sb[:, et, :], xT_sb[:, et, lo:lo + 512],
                                 start=(et == 0), stop=(et == ET - 1))
            nc.vector.tensor_scalar(out=vT_sb[:, lo:lo + 512], in0=pv[:],
                                    scalar1=bv_sb[:, 0:1], scalar2=None, op0=ADD)
            for ktl in range(4):
                kt = sc * 4 + ktl
                ptr = ps.tile([128, 128], BF16, tag=("y" if ktl % 2 else "misc"),
                              name=f"ptr{kt}")
                nc.tensor.transpose(ptr[:], vT_sb[:, kt * 128:(kt + 1) * 128], ident[:])
                for g in range(2):
                    nc.vector.tensor_copy(
                        out=vaug[:, g * KT + kt, 0:64], in_=ptr[:, g * 64:(g + 1) * 64])

        # ---- helpers issued inline or as fillers ----
        def qproj_half(sc, t, half):
            """Half of one qT tile's 8-matmul accumulation (4 MMs)."""
            lo = sc * 512
            if half == 0:
                pq = ps.tile([128, 512], F32, tag="misc", name=f"pq{sc}_{t}")
                _qp_tiles[(sc, t)] = pq
            else:
                pq = _qp_tiles[(sc, t)]
            for et in range(4 * half, 4 * half + 4):
                nc.tensor.matmul(pq[:], wq_sb[:, et, t * 128:(t + 1) * 128],
                                 xT_sb[:, et, lo:lo + 512],
                                 start=(et == 0), stop=(et == ET - 1))
            if half == 1:
                nc.vector.tensor_scalar(
                    out=qT_sb[:, t, lo:lo + 512], in0=pq[:],
                    scalar1=bq_sb[:, t:t + 1], scalar2=None, op0=ADD)

        _qp_tiles = {}

        def outproj_et(qc, et, avT_t):
            lo = qc * 512
            yp = ps.tile([128, 512], F32, tag="y", name=f"yp{qc}_{et}")
            for t in range(QT):
                nc.tensor.matmul(yp[:], wo_sb[:, t, et * 128:(et + 1) * 128],
                                 avT_t[:, t, :], start=(t == 0), stop=(t == QT - 1))
            ysb = ypool.tile([128, 512], F32, tag="ysb", name=f"ysb{qc}_{et}")
            nc.vector.tensor_copy(out=ysb[:], in_=yp[:])
            nc.sync.dma_start(out=yT[et * 128:(et + 1) * 128, lo:lo + 512],
                              in_=ysb[:])

        # Q projection for chunk 0 runs up front.
        for t in range(QT):
            qproj_half(0, t, 0)
            qproj_half(0, t, 1)

        # ================= phase 2: attention, software-pipelined =================
        avT_tiles = {}
        for qc in range(SC):
            lo = qc * 512
            fillers = deque()
            if qc + 1 < SC:
                for t in range(QT):
                    fillers.append(lambda t=t: qproj_half(qc + 1, t, 0))
                    fillers.append(lambda t=t: qproj_half(qc + 1, t, 1))
            if qc - 1 >= 0:
                prev_avT = avT_tiles[qc - 1]
                for et in range(ET):
                    fillers.append(lambda et=et, a=prev_avT: outproj_et(qc - 1, et, a))

            avT_t = avtpool.tile([128, QT, 512], BF16, tag="avT", name=f"avT{qc}")
            avT_tiles[qc] = avT_t
            for p in range(QT):
                avpA = ps.tile([128, 512], F32, tag="avA", name=f"avpA{qc}_{p}")
                avpB = ps.tile([128, 512], F32, tag="avB", name=f"avpB{qc}_{p}")
                for j in range(KT // 2):
                    kt0 = 2 * j
                    # scores: [A kt0 | A kt1 | B kt0 | B kt1] in one 4-bank span.
                    # A (rows 0-63) and B (rows 64-127) overlap in the PE array.
                    sAB = ps.tile([128, 2048], F32, tag="s", name=f"s{qc}_{p}_{j}")
                    for idx in range(2):
                        kt = kt0 + idx
                        nc.tensor.matmul(
                            sAB[:, idx * 512:(idx + 1) * 512],
                            kT_sb[0:64, kt * 128:(kt + 1) * 128],
                            qT_sb[0:64, p, lo:lo + 512], start=True, stop=True)
                        nc.tensor.matmul(
                            sAB[:, (2 + idx) * 512:(3 + idx) * 512],
                            kT_sb[64:128, kt * 128:(kt + 1) * 128],
                            qT_sb[64:128, p, lo:lo + 512], start=True, stop=True)
                    pe_t = pepool.tile([128, 2048], BF16, tag="pe",
                                       name=f"pe{qc}_{p}_{j}")
                    nc.scalar.activation(pe_t[:], sAB[:], EXP, scale=float(SCALE))
                    if fillers:
                        fillers.popleft()()
                    for idx in range(2):
                        kt = kt0 + idx
                        nc.tensor.matmul(
                            avpA[0:65, :], vaug[:, kt, :],
                            pe_t[:, idx * 512:(idx + 1) * 512],
                            start=(kt == 0), stop=(kt == KT - 1))
                        nc.tensor.matmul(
                            avpB[0:65, :], vaug[:, KT + kt, :],
                            pe_t[:, (2 + idx) * 512:(3 + idx) * 512],
                            start=(kt == 0), stop=(kt == KT - 1))
                # normalize: avT = av[0:64] * recip(av[64]), denom replicated
                for g, avp in ((0, avpA), (1, avpB)):
                    ph = g * 64
                    linv = npool.tile([1, 512], BF16, tag="linv",
                                      name=f"linv{qc}_{p}_{g}")
                    nc.vector.reciprocal(linv[:], avp[64:65, :])
                    lrp = ps.tile([128, 512], F32, tag="misc",
                                  name=f"lrp{qc}_{p}_{g}")
                    nc.tensor.matmul(lrp[0:64, :], ones_row[:], linv[:],
                                     start=True, stop=True)
                    lrep = npool.tile([64, 512], F32, tag="lrep",
                                      name=f"lrep{qc}_{p}_{g}")
                    nc.vector.tensor_copy(out=lrep[:], in_=lrp[0:64, :])
                    nc.vector.tensor_tensor(
                        out=avT_t[ph:ph + 64, p, :], in0=avp[0:64, :], in1=lrep[:],
                        op=MUL)
                if fillers:
                    fillers.popleft()()
            while fillers:
                fillers.popleft()()
        # out-projection for the last chunk
        for et in range(ET):
            outproj_et(SC - 1, et, avT_tiles[SC - 1])
    nc.compile()
    return nc


def _shard_inputs(x, Wq, bq, Wk, bk, Wv, bv, Wo, bo):
    """Build the 8 per-core input maps (bf16 weights/activations)."""
    x = np.asarray(x, dtype=np.float32)
    in_maps = []
    for c in range(8):
        b, H = c // 2, c % 2
        heads = [8 * H + t for t in range(4)] + [8 * H + t + 4 for t in range(4)]
        # qT tile t holds (local head t -> partitions 0-63, local head t+4 -> 64-127)
        order = []
        for t in range(4):
            order.extend(range(heads[t] * 64, heads[t] * 64 + 64))
            order.extend(range(heads[t + 4] * 64, heads[t + 4] * 64 + 64))
        order = np.asarray(order)
        wq_p = np.ascontiguousarray(np.asarray(Wq, np.float32)[:, order]).astype(NPBF16)
        bq_p = np.ascontiguousarray(
            np.asarray(bq, np.float32)[order].reshape(4, 128).T)
        wo_p = np.ascontiguousarray(np.asarray(Wo, np.float32)[order, :]).astype(NPBF16)
        wk_s = np.ascontiguousarray(
            np.asarray(Wk, np.float32)[:, H * 128:(H + 1) * 128]).astype(NPBF16)
        wv_s = np.ascontiguousarray(
            np.asarray(Wv, np.float32)[:, H * 128:(H + 1) * 128]).astype(NPBF16)
        bk_s = np.ascontiguousarray(np.asarray(bk, np.float32)[H * 128:(H + 1) * 128]
                                    .reshape(128, 1))
        bv_s = np.ascontiguousarray(np.asarray(bv, np.float32)[H * 128:(H + 1) * 128]
                                    .reshape(128, 1))
        xT_b = np.ascontiguousarray(x[b].T.astype(NPBF16))
        in_maps.append({
            "xT": xT_b, "wq": wq_p, "wk": wk_s, "wv": wv_s, "wo": wo_p,
            "bqd": bq_p, "bkd": bk_s, "bvd": bv_s,
            "identd": np.eye(128, dtype=NPBF16),
            "onesd": np.ones((128, 64), dtype=NPBF16),
        })
    return in_maps


def kernel(x, Wq, bq, Wk, bk, Wv, bv, Wo, bo, _trace=False):
    if "nc" not in _NC_CACHE:
        _NC_CACHE["nc"] = build_nc()
    nc = _NC_CACHE["nc"]
    in_maps = _shard_inputs(x, Wq, bq, Wk, bk, Wv, bv, Wo, bo)
    res = run_bass_kernel_spmd(nc, in_maps, list(range(8)), trace=_trace)
    bo = np.asarray(bo, dtype=np.float32)
    out = np.empty((B, S, E), dtype=np.float32)
    for b in range(B):
        yT = res.results[2 * b]["yT"] + res.results[2 * b + 1]["yT"]
        out[b] = yT.T + bo
    if _trace:
        return out, res
    return out


# revision 8
# speedup vs baseline: 2.0135x; 1.3331x over previous
"""GQA self-attention kernel for Trainium2, sharded over 8 NeuronCores.

Problem: x[4, 2048, 1024], 16 heads / 4 KV groups / head_dim 64.
Sharding: batch (4) x head-half (2 KV groups each) = 8 cores.

v2 dataflow (all-bf16 PE path, transposed world: features on partitions):
  xT[1024,2048]b16 -> qT[512,2048], kT[128,2048], vT[128,2048]  (PE, bf16)
  vT --PE transpose--> vaug[seq,65] tiles (ones col appended -> softmax sums)
  scores s[k,q] = kT_g^T(d,kpos) . qT_h(d,q): K=64 contraction, the two KV
    groups run CONCURRENTLY in the upper/lower 64-row halves of the PE array
    (row tiling via base_partition-derived tile_position)
  exp: one ACT instruction per [128,2048] PSUM span (2 kt tiles x 2 heads)
    -> bf16 p tiles; amortizes the ~352-cycle ACT fixed cost
  av[65,q] += vaug^T p (row 64 = softmax denominator)
  avT_norm = av[0:64] * recip(av[64])  (DVE; denom row replicated via K=1 MM)
  yT[e,q] = Wo_p^T . avT_norm -> DRAM (f32)
The attention loop is software-pipelined; Q-projection (next chunk) and
out-projection (previous chunk) matmuls are spliced into the PE queue as
fillers during exp waits so the PE stays dense (HAM stays warm).
Host: y[b] = (yT[2b] + yT[2b+1]).T + bo
"""

import sys
import numpy as np

sys.path.insert(0, "/opt/trn_rl_repo")

from collections import deque
from contextlib import ExitStack

import ml_dtypes

import concourse.bass as bass
import concourse.bacc as bacc
import concourse.mybir as mybir
from concourse import tile
from concourse.bass_utils import run_bass_kernel_spmd

F32 = mybir.dt.float32
BF16 = mybir.dt.bfloat16
NPBF16 = ml_dtypes.bfloat16

B, S, E = 4, 2048, 1024
NUM_HEADS, NUM_GROUPS, D = 16, 4, 64
CQ = 512          # q cols per core (8 heads)
CK = 128          # kv cols per core (2 groups)
ET = E // 128     # 8 embed K-tiles
SC = S // 512     # 4 seq chunks of 512
KT = S // 128     # 16 key tiles of 128
QT = CQ // 128    # 4 qT partition tiles (pair p: head p @0-63, head p+4 @64-127)
SCALE = 1.0 / np.sqrt(np.float32(D))

_NC_CACHE = {}


def build_nc():
    nc = bacc.Bacc(None, target_bir_lowering=False)

    xT = nc.dram_tensor("xT", [E, S], BF16, kind="ExternalInput")
    wq = nc.dram_tensor("wq", [E, CQ], BF16, kind="ExternalInput")
    wk = nc.dram_tensor("wk", [E, CK], BF16, kind="ExternalInput")
    wv = nc.dram_tensor("wv", [E, CK], BF16, kind="ExternalInput")
    wo = nc.dram_tensor("wo", [CQ, E], BF16, kind="ExternalInput")
    bqd = nc.dram_tensor("bqd", [128, QT], F32, kind="ExternalInput")
    bkd = nc.dram_tensor("bkd", [128, 1], F32, kind="ExternalInput")
    bvd = nc.dram_tensor("bvd", [128, 1], F32, kind="ExternalInput")
    identd = nc.dram_tensor("identd", [128, 128], BF16, kind="ExternalInput")
    onesd = nc.dram_tensor("onesd", [128, 64], BF16, kind="ExternalInput")
    onesf = nc.dram_tensor("onesf", [1, 64], mybir.dt.float32r, kind="ExternalInput")
    yT = nc.dram_tensor("yT", [E, S], F32, kind="ExternalOutput")

    ADD = mybir.AluOpType.add
    MUL = mybir.AluOpType.mult
    EXP = mybir.ActivationFunctionType.Exp

    with tile.TileContext(nc) as tc, ExitStack() as ctx, \
            nc.allow_low_precision(reason="bf16 matmuls within 2e-2 tolerance"):
        const = ctx.enter_context(tc.tile_pool(name="const", bufs=1))
        wpool = ctx.enter_context(tc.tile_pool(name="wpool", bufs=1))
        big = ctx.enter_context(tc.tile_pool(name="big", bufs=1))
        pepool = ctx.enter_context(tc.tile_pool(name="pepool", bufs=4))
        avtpool = ctx.enter_context(tc.tile_pool(name="avtpool", bufs=2))
        npool = ctx.enter_context(tc.tile_pool(name="npool", bufs=2))
        ypool = ctx.enter_context(tc.tile_pool(name="ypool", bufs=2))
        # PSUM budget (16KB/partition = 8 banks, exact fit):
        #   s    [128,2048] f32  4 banks  (scores: 2 kt x 2 heads per j)
        #   avA  [128, 512] f32  1 bank   (head A attention accumulator)
        #   avB  [128, 512] f32  1 bank
        #   y    [128, 512] f32  1 bank   (out-proj + phase1 rotation)
        #   misc [128, 512] f32  1 bank   (q-proj fillers, lrp, transposes)
        ps = ctx.enter_context(tc.tile_pool(name="ps", bufs=1, space="PSUM"))

        # ---- constants ----
        ident = const.tile([128, 128], BF16)
        nc.sync.dma_start(out=ident[:], in_=identd[:, :])
        ones_row = const.tile([1, 64], mybir.dt.float32r)
        nc.sync.dma_start(out=ones_row[:], in_=onesf[0:1, :])

        # ---- weights + biases (kv first: phase 1 needs them immediately) ----
        wk_sb = wpool.tile([128, ET, CK], BF16)
        wv_sb = wpool.tile([128, ET, CK], BF16)
        for et in range(ET):
            nc.sync.dma_start(out=wk_sb[:, et, :], in_=wk[et * 128:(et + 1) * 128, :])
            nc.sync.dma_start(out=wv_sb[:, et, :], in_=wv[et * 128:(et + 1) * 128, :])
        bk_sb = wpool.tile([128, 1], F32)
        nc.sync.dma_start(out=bk_sb[:], in_=bkd[:, :])
        bv_sb = wpool.tile([128, 1], F32)
        nc.sync.dma_start(out=bv_sb[:], in_=bvd[:, :])
        wq_sb = wpool.tile([128, ET, CQ], BF16)
        for et in range(ET):
            nc.sync.dma_start(out=wq_sb[:, et, :], in_=wq[et * 128:(et + 1) * 128, :])
        bq_sb = wpool.tile([128, QT], F32)
        nc.sync.dma_start(out=bq_sb[:], in_=bqd[:, :])
        wo_sb = wpool.tile([128, QT, E], BF16)
        for t in range(QT):
            nc.sync.dma_start(out=wo_sb[:, t, :], in_=wo[t * 128:(t + 1) * 128, :])

        # ---- persistent activations ----
        xT_sb = big.tile([128, ET, S], BF16)      # 32KB/partition
        for sc in range(SC):
            lo = sc * 512
            for et in range(ET):
                nc.sync.dma_start(
                    out=xT_sb[:, et, lo:lo + 512],
                    in_=xT[et * 128:(et + 1) * 128, lo:lo + 512])
        qT_sb = big.tile([128, QT, S], BF16)      # 16KB
        kT_sb = big.tile([128, S], BF16)          # 4KB
        vT_sb = big.tile([128, S], BF16)          # 4KB
        vaug = big.tile([128, 2 * KT, 65], BF16)  # v natural + ones col
        for g in range(2):
            for kt in range(KT):
                nc.sync.dma_start(out=vaug[:, g * KT + kt, 64:65],
                                  in_=onesd[:, 0:1])

        # ================= phase 1: K/V projections + transposes =================
        for sc in range(SC):
            lo = sc * 512
            pk = ps.tile([128, 512], F32, tag="avA", name=f"pk{sc}")
            for et in range(ET):
                nc.tensor.matmul(pk[:], wk_sb[:, et, :], xT_sb[:, et, lo:lo + 512],
                                 start=(et == 0), stop=(et == ET - 1))
            nc.vector.tensor_scalar(out=kT_sb[:, lo:lo + 512], in0=pk[:],
                                    scalar1=bk_sb[:, 0:1], scalar2=None, op0=ADD)
            pv = ps.tile([128, 512], F32, tag="avB", name=f"pv{sc}")
            for et in range(ET):
                nc.tensor.matmul(pv[:], wv_sb[:, et, :], xT_sb[:, et, lo:lo + 512],
                                 start=(et == 0), stop=(et == ET - 1))
            nc.vector.tensor_scalar(out=vT_sb[:, lo:lo + 512], in0=pv[:],
                                    scalar1=bv_sb[:, 0:1], scalar2=None, op0=ADD)
            for ktl in range(4):
                kt = sc * 4 + ktl
                ptr = ps.tile([128, 128], BF16, tag=("y" if ktl % 2 else "misc"),
                              name=f"ptr{kt}")
                nc.tensor.transpose(ptr[:], vT_sb[:, kt * 128:(kt + 1) * 128], ident[:])
                for g in range(2):
                    nc.vector.tensor_copy(
                        out=vaug[:, g * KT + kt, 0:64], in_=ptr[:, g * 64:(g + 1) * 64])

        # ---- helpers issued inline or as fillers ----
        def qproj_half(sc, t, half):
            """Half of one qT tile's 8-matmul accumulation (4 MMs)."""
            lo = sc * 512
            if half == 0:
                pq = ps.tile([128, 512], F32, tag="misc", name=f"pq{sc}_{t}")
                _qp_tiles[(sc, t)] = pq
            else:
                pq = _qp_tiles[(sc, t)]
            for et in range(4 * half, 4 * half + 4):
                nc.tensor.matmul(pq[:], wq_sb[:, et, t * 128:(t + 1) * 128],
                                 xT_sb[:, et, lo:lo + 512],
                                 start=(et == 0), stop=(et == ET - 1))
            if half == 1:
                nc.vector.tensor_scalar(
                    out=qT_sb[:, t, lo:lo + 512], in0=pq[:],
                    scalar1=bq_sb[:, t:t + 1], scalar2=None, op0=ADD)

        _qp_tiles = {}

        def outproj_et(qc, et, avT_t):
            lo = qc * 512
            yp = ps.tile([128, 512], F32, tag="y", name=f"yp{qc}_{et}")
            for t in range(QT):
                nc.tensor.matmul(yp[:], wo_sb[:, t, et * 128:(et + 1) * 128],
                                 avT_t[:, t, :], start=(t == 0), stop=(t == QT - 1))
            ysb = ypool.tile([128, 512], F32, tag="ysb", name=f"ysb{qc}_{et}")
            nc.vector.tensor_copy(out=ysb[:], in_=yp[:])
            nc.sync.dma_start(out=yT[et * 128:(et + 1) * 128, lo:lo + 512],
                              in_=ysb[:])

        # Q projection for chunk 0 runs up front.
        for t in range(QT):
            qproj_half(0, t, 0)
            qproj_half(0, t, 1)

        # ================= phase 2: attention, software-pipelined =================
        def normalize(avp, avT_t, p, g, qc):
            """avT = av[0:64] * recip(av[64]); denom row replicated via K=1 MM."""
            ph = g * 64
            linv = npool.tile([1, 512], mybir.dt.float32r, tag="linv",
                              name=f"linv{qc}_{p}_{g}")
            nc.vector.reciprocal(linv[:], avp[64:65, :])
            lrp = ps.tile([128, 512], F32, tag="y", name=f"lrp{qc}_{p}_{g}")
            nc.tensor.matmul(lrp[0:64, :], ones_row[:], linv[:],
                             start=True, stop=True)
            lrep = npool.tile([64, 512], F32, tag="lrep", name=f"lrep{qc}_{p}_{g}")
            nc.vector.tensor_copy(out=lrep[:], in_=lrp[0:64, :])
            nc.vector.tensor_tensor(
                out=avT_t[ph:ph + 64, p, :], in0=avp[0:64, :], in1=lrep[:], op=MUL)

        avT_tiles = {}
        pending_norm = deque()  # deferred normalizes, popped after next S/exp
        for qc in range(SC):
            lo = qc * 512
            fillers = deque()
            if qc + 1 < SC:
                for t in range(QT):
                    fillers.append(lambda t=t, s=qc + 1: qproj_half(s, t, 0))
                    fillers.append(lambda t=t, s=qc + 1: qproj_half(s, t, 1))
            if qc - 1 >= 0:
                prev_avT = avT_tiles[qc - 1]
                for et in range(ET):
                    fillers.append(lambda et=et, a=prev_avT, s=qc - 1:
                                   outproj_et(s, et, a))

            avT_t = avtpool.tile([128, QT, 512], BF16, tag="avT", name=f"avT{qc}")
            avT_tiles[qc] = avT_t
            for p in range(QT):
                # scores/exp run one kt ahead of AV; pair (p-1)'s normalize is
                # spliced in after this pair's first exp so its PE/DVE ops hide
                # under the exp stream instead of stalling the pair boundary.
                avpA = avpB = None
                pe_tiles = {}
                for kt in range(KT):
                    sT = ps.tile([128, 1024], F32, tag="s", bufs=2,
                                 name=f"s{qc}_{p}_{kt}")
                    nc.tensor.matmul(
                        sT[:, 0:512],
                        kT_sb[0:64, kt * 128:(kt + 1) * 128],
                        qT_sb[0:64, p, lo:lo + 512], start=True, stop=True)
                    nc.tensor.matmul(
                        sT[:, 512:1024],
                        kT_sb[64:128, kt * 128:(kt + 1) * 128],
                        qT_sb[64:128, p, lo:lo + 512], start=True, stop=True)
                    pe_t = pepool.tile([128, 1024], BF16, tag="pe",
                                       name=f"pe{qc}_{p}_{kt}")
                    nc.scalar.activation(pe_t[:], sT[:], EXP, scale=float(SCALE))
                    pe_tiles[kt] = pe_t
                    if kt == 1:
                        while pending_norm:
                            pending_norm.popleft()()
                        # allocate accumulators after the deferred normalize of
                        # the previous pair has issued its reads (bufs=1 slots)
                        avpA = ps.tile([128, 512], F32, tag="avA",
                                       name=f"avpA{qc}_{p}")
                        avpB = ps.tile([128, 512], F32, tag="avB",
                                       name=f"avpB{qc}_{p}")
                    if kt >= 1:
                        pkt = kt - 1
                        pp = pe_tiles.pop(pkt)
                        nc.tensor.matmul(
                            avpA[0:65, :], vaug[:, pkt, :], pp[:, 0:512],
                            start=(pkt == 0), stop=False)
                        nc.tensor.matmul(
                            avpB[0:65, :], vaug[:, KT + pkt, :], pp[:, 512:1024],
                            start=(pkt == 0), stop=False)
                    if kt >= 3 and kt % 2 == 1 and fillers:
                        fillers.popleft()()
                pp = pe_tiles.pop(KT - 1)
                nc.tensor.matmul(avpA[0:65, :], vaug[:, KT - 1, :], pp[:, 0:512],
                                 start=False, stop=True)
                nc.tensor.matmul(avpB[0:65, :], vaug[:, 2 * KT - 1, :],
                                 pp[:, 512:1024], start=False, stop=True)
                pending_norm.append(
                    lambda a=avpA, t=avT_t, p=p, q=qc: normalize(a, t, p, 0, q))
                pending_norm.append(
                    lambda a=avpB, t=avT_t, p=p, q=qc: normalize(a, t, p, 1, q))
            while fillers:
                fillers.popleft()()
        while pending_norm:
            pending_norm.popleft()()
        # out-projection for the last chunk
        for et in range(ET):
            outproj_et(SC - 1, et, avT_tiles[SC - 1])
    nc.compile()
    return nc


def _shard_inputs(x, Wq, bq, Wk, bk, Wv, bv, Wo, bo):
    """Build the 8 per-core input maps (bf16 weights/activations)."""
    x = np.asarray(x, dtype=np.float32)
    in_maps = []
    for c in range(8):
        b, H = c // 2, c % 2
        heads = [8 * H + t for t in range(4)] + [8 * H + t + 4 for t in range(4)]
        # qT tile t holds (local head t -> partitions 0-63, local head t+4 -> 64-127)
        order = []
        for t in range(4):
            order.extend(range(heads[t] * 64, heads[t] * 64 + 64))
            order.extend(range(heads[t + 4] * 64, heads[t + 4] * 64 + 64))
        order = np.asarray(order)
        wq_p = np.ascontiguousarray(np.asarray(Wq, np.float32)[:, order]).astype(NPBF16)
        bq_p = np.ascontiguousarray(
            np.asarray(bq, np.float32)[order].reshape(4, 128).T)
        wo_p = np.ascontiguousarray(np.asarray(Wo, np.float32)[order, :]).astype(NPBF16)
        wk_s = np.ascontiguousarray(
            np.asarray(Wk, np.float32)[:, H * 128:(H + 1) * 128]).astype(NPBF16)
        wv_s = np.ascontiguousarray(
            np.asarray(Wv, np.float32)[:, H * 128:(H + 1) * 128]).astype(NPBF16)
        bk_s = np.ascontiguousarray(np.asarray(bk, np.float32)[H * 128:(H + 1) * 128]
                                    .reshape(128, 1))
        bv_s = np.ascontiguousarray(np.asarray(bv, np.float32)[H * 128:(H + 1) * 128]
                                    .reshape(128, 1))
        xT_b = np.ascontiguousarray(x[b].T.astype(NPBF16))
        in_maps.append({
            "xT": xT_b, "wq": wq_p, "wk": wk_s, "wv": wv_s, "wo": wo_p,
            "bqd": bq_p, "bkd": bk_s, "bvd": bv_s,
            "identd": np.eye(128, dtype=NPBF16),
            "onesd": np.ones((128, 64), dtype=NPBF16),
            "onesf": np.ones((1, 64), dtype=np.float32),
        })
    return in_maps


def kernel(x, Wq, bq, Wk, bk, Wv, bv, Wo, bo, _trace=False):
    if "nc" not in _NC_CACHE:
        _NC_CACHE["nc"] = build_nc()
    nc = _NC_CACHE["nc"]
    in_maps = _shard_inputs(x, Wq, bq, Wk, bk, Wv, bv, Wo, bo)
    res = run_bass_kernel_spmd(nc, in_maps, list(range(8)), trace=_trace)
    bo = np.asarray(bo, dtype=np.float32)
    out = np.empty((B, S, E), dtype=np.float32)
    for b in range(B):
        yT = res.results[2 * b]["yT"] + res.results[2 * b + 1]["yT"]
        out[b] = yT.T + bo
    if _trace:
        return out, res
    return out


# revision 47
# speedup vs baseline: 2.0500x; 1.0181x over previous
"""GQA self-attention kernel for Trainium2, sharded over 8 NeuronCores.

Problem: x[4, 2048, 1024], 16 heads / 4 KV groups / head_dim 64.
Sharding: batch (4) x head-half (2 KV groups each) = 8 cores.

v2 dataflow (all-bf16 PE path, transposed world: features on partitions):
  xT[1024,2048]b16 -> qT[512,2048], kT[128,2048], vT[128,2048]  (PE, bf16)
  vT --PE transpose--> vaug[seq,65] tiles (ones col appended -> softmax sums)
  scores s[k,q] = kT_g^T(d,kpos) . qT_h(d,q): K=64 contraction, the two KV
    groups run CONCURRENTLY in the upper/lower 64-row halves of the PE array
    (row tiling via base_partition-derived tile_position)
  exp: one ACT instruction per [128,2048] PSUM span (2 kt tiles x 2 heads)
    -> bf16 p tiles; amortizes the ~352-cycle ACT fixed cost
  av[65,q] += vaug^T p (row 64 = softmax denominator)
  avT_norm = av[0:64] * recip(av[64])  (DVE; denom row replicated via K=1 MM)
  yT[e,q] = Wo_p^T . avT_norm -> DRAM (f32)
The attention loop is software-pipelined; Q-projection (next chunk) and
out-projection (previous chunk) matmuls are spliced into the PE queue as
fillers during exp waits so the PE stays dense (HAM stays warm).
Host: y[b] = (yT[2b] + yT[2b+1]).T + bo
"""

import os
import sys
import numpy as np

# Coarse whole-tile dependency tracking: subtile AP-overlap analysis has
# proven racy for this kernel's 3D-sliced persistent tiles (intermittent
# first-run corruption); whole-tile deps are conservative and safe.
os.environ["BY_DEFAULT_DISABLE_SUBTILE_DEPS"] = "1"

sys.path.insert(0, "/opt/trn_rl_repo")

from collections import deque
from contextlib import ExitStack

import ml_dtypes

import concourse.bass as bass
import concourse.bacc as bacc
import concourse.mybir as mybir
from concourse import tile
from concourse.bass_utils import run_bass_kernel_spmd

F32 = mybir.dt.float32
BF16 = mybir.dt.bfloat16
NPBF16 = ml_dtypes.bfloat16

B, S, E = 4, 2048, 1024
NUM_HEADS, NUM_GROUPS, D = 16, 4, 64
CQ = 512          # q cols per core (8 heads)
CK = 128          # kv cols per core (2 groups)
ET = E // 128     # 8 embed K-tiles
SC = S // 512     # 4 seq chunks of 512
KT = S // 128     # 16 key tiles of 128
QT = CQ // 128    # 4 qT partition tiles (pair p: head p @0-63, head p+4 @64-127)
SCALE = 1.0 / np.sqrt(np.float32(D))
# Schraudolph fast-exp constants emitting bf16 bits as int16:
#   bf16_bits(e^(x*SCALE)) ~= int16((2^23/ln2)*SCALE/2^16 * x + (127*2^23-c)/2^16)
# c = 486411 minimizes rms relative error (~1.8%); used on a minority of
# attention kt-tiles (GPSIMD/DVE) to offload the ACT-bound exp stream.
SCHR_A = float((2.0 ** 23 / np.log(2.0)) * SCALE / 65536.0)
SCHR_B = float((127 * 2 ** 23 - 486411) / 65536.0)
GPS_KTS = ()              # GPSIMD cannot read PSUM (walrus birverifier)
DVE_KTS = (3, 7, 11, 15)  # exp tiles computed on DVE (25% offload)

_NC_CACHE = {}


def build_nc():
    nc = bacc.Bacc(None, target_bir_lowering=False)

    xT = nc.dram_tensor("xT", [E, S], BF16, kind="ExternalInput")
    wq = nc.dram_tensor("wq", [E, CQ], BF16, kind="ExternalInput")
    wk = nc.dram_tensor("wk", [E, CK], BF16, kind="ExternalInput")
    wv = nc.dram_tensor("wv", [E, CK], BF16, kind="ExternalInput")
    wo = nc.dram_tensor("wo", [CQ, E], BF16, kind="ExternalInput")
    bqd = nc.dram_tensor("bqd", [128, QT], F32, kind="ExternalInput")
    bkd = nc.dram_tensor("bkd", [128, 1], F32, kind="ExternalInput")
    bvd = nc.dram_tensor("bvd", [128, 1], F32, kind="ExternalInput")
    identd = nc.dram_tensor("identd", [128, 128], BF16, kind="ExternalInput")
    onesd = nc.dram_tensor("onesd", [128, 64], BF16, kind="ExternalInput")
    onesf = nc.dram_tensor("onesf", [1, 64], mybir.dt.float32r, kind="ExternalInput")
    yT = nc.dram_tensor("yT", [E, S], F32, kind="ExternalOutput")

    ADD = mybir.AluOpType.add
    MUL = mybir.AluOpType.mult
    EXP = mybir.ActivationFunctionType.Exp

    with tile.TileContext(nc) as tc, ExitStack() as ctx, \
            nc.allow_low_precision(reason="bf16 matmuls within 2e-2 tolerance"):
        const = ctx.enter_context(tc.tile_pool(name="const", bufs=1))
        wpool = ctx.enter_context(tc.tile_pool(name="wpool", bufs=1))
        big = ctx.enter_context(tc.tile_pool(name="big", bufs=1))
        pepool = ctx.enter_context(tc.tile_pool(name="pepool", bufs=4))
        avtpool = ctx.enter_context(tc.tile_pool(name="avtpool", bufs=2))
        npool = ctx.enter_context(tc.tile_pool(name="npool", bufs=2))
        ypool = ctx.enter_context(tc.tile_pool(name="ypool", bufs=2))
        # PSUM budget (16KB/partition = 8 banks, exact fit):
        #   s    [128,2048] f32  4 banks  (scores: 2 kt x 2 heads per j)
        #   avA  [128, 512] f32  1 bank   (head A attention accumulator)
        #   avB  [128, 512] f32  1 bank
        #   y    [128, 512] f32  1 bank   (out-proj + phase1 rotation)
        #   misc [128, 512] f32  1 bank   (q-proj fillers, lrp, transposes)
        ps = ctx.enter_context(tc.tile_pool(name="ps", bufs=1, space="PSUM"))

        # ---- constants ----
        ident = const.tile([128, 128], BF16)
        nc.sync.dma_start(out=ident[:], in_=identd[:, :])
        ones_row = const.tile([1, 64], mybir.dt.float32r)
        nc.sync.dma_start(out=ones_row[:], in_=onesf[0:1, :])

        # ---- weights + biases (kv + x chunk0 first: needed immediately) ----
        wk_sb = wpool.tile([128, ET, CK], BF16)
        wv_sb = wpool.tile([128, ET, CK], BF16)
        for et in range(ET):
            nc.sync.dma_start(out=wk_sb[:, et, :], in_=wk[et * 128:(et + 1) * 128, :])
            nc.sync.dma_start(out=wv_sb[:, et, :], in_=wv[et * 128:(et + 1) * 128, :])
        bk_sb = wpool.tile([128, 1], F32)
        nc.sync.dma_start(out=bk_sb[:], in_=bkd[:, :])
        bv_sb = wpool.tile([128, 1], F32)
        nc.sync.dma_start(out=bv_sb[:], in_=bvd[:, :])

        xT_sb = big.tile([128, ET, S], BF16)      # 32KB/partition
        qT_sb = big.tile([128, QT, S], BF16)      # 16KB
        kT_sb = big.tile([128, S], BF16)          # 4KB
        vT_sb = big.tile([128, S], BF16)          # 4KB
        vaug = big.tile([128, 2 * KT, 65], BF16)  # v natural + ones col

        for et in range(ET):
            nc.sync.dma_start(out=xT_sb[:, et, 0:512],
                              in_=xT[et * 128:(et + 1) * 128, 0:512])
        wq_sb = wpool.tile([128, ET, CQ], BF16)
        for et in range(ET):
            nc.sync.dma_start(out=wq_sb[:, et, :], in_=wq[et * 128:(et + 1) * 128, :])
        bq_sb = wpool.tile([128, QT], F32)
        nc.sync.dma_start(out=bq_sb[:], in_=bqd[:, :])
        for g in range(2):
            for kt in range(KT):
                nc.sync.dma_start(out=vaug[:, g * KT + kt, 64:65],
                                  in_=onesd[:, 0:1])
        for sc in range(1, SC):
            lo = sc * 512
            for et in range(ET):
                nc.sync.dma_start(
                    out=xT_sb[:, et, lo:lo + 512],
                    in_=xT[et * 128:(et + 1) * 128, lo:lo + 512])
        wo_sb = wpool.tile([128, QT, E], BF16)
        for t in range(QT):
            nc.sync.dma_start(out=wo_sb[:, t, :], in_=wo[t * 128:(t + 1) * 128, :])

        # ================= phase 1: K/V projections + transposes =================
        def kv_proj(sc):
            lo = sc * 512
            pk = ps.tile([128, 512], F32, tag="y", name=f"pk{sc}")
            for et in range(ET):
                nc.tensor.matmul(pk[:], wk_sb[:, et, :], xT_sb[:, et, lo:lo + 512],
                                 start=(et == 0), stop=(et == ET - 1))
            nc.vector.tensor_scalar(out=kT_sb[:, lo:lo + 512], in0=pk[:],
                                    scalar1=bk_sb[:, 0:1], scalar2=None, op0=ADD)
            pv = ps.tile([128, 512], F32, tag="misc", name=f"pv{sc}")
            for et in range(ET):
                nc.tensor.matmul(pv[:], wv_sb[:, et, :], xT_sb[:, et, lo:lo + 512],
                                 start=(et == 0), stop=(et == ET - 1))
            nc.vector.tensor_scalar(out=vT_sb[:, lo:lo + 512], in0=pv[:],
                                    scalar1=bv_sb[:, 0:1], scalar2=None, op0=ADD)
            for ktl in range(4):
                kt = sc * 4 + ktl
                ptr = ps.tile([128, 128], BF16, tag="y", name=f"ptr{kt}")
                nc.tensor.transpose(ptr[:], vT_sb[:, kt * 128:(kt + 1) * 128], ident[:])
                for g in range(2):
                    nc.vector.tensor_copy(
                        out=vaug[:, g * KT + kt, 0:64], in_=ptr[:, g * 64:(g + 1) * 64])

        for sc in range(SC):
            kv_proj(sc)

        # ---- helpers issued inline or as fillers ----
        def qproj(sc, t):
            """One qT tile: 8-matmul accumulation + bias, issued atomically
            (tag-rotation safety: nothing else may allocate this tag between
            a tile's first write and its last read)."""
            lo = sc * 512
            pq = ps.tile([128, 512], F32, tag="misc", name=f"pq{sc}_{t}")
            for et in range(ET):
                nc.tensor.matmul(pq[:], wq_sb[:, et, t * 128:(t + 1) * 128],
                                 xT_sb[:, et, lo:lo + 512],
                                 start=(et == 0), stop=(et == ET - 1))
            nc.vector.tensor_scalar(
                out=qT_sb[:, t, lo:lo + 512], in0=pq[:],
                scalar1=bq_sb[:, t:t + 1], scalar2=None, op0=ADD)

        def outproj_et(qc, et, avT_t):
            lo = qc * 512
            yp = ps.tile([128, 512], F32, tag="y", name=f"yp{qc}_{et}")
            for t in range(QT):
                nc.tensor.matmul(yp[:], wo_sb[:, t, et * 128:(et + 1) * 128],
                                 avT_t[:, t, :], start=(t == 0), stop=(t == QT - 1))
            ysb = ypool.tile([128, 512], F32, tag="ysb", name=f"ysb{qc}_{et}")
            nc.vector.tensor_copy(out=ysb[:], in_=yp[:])
            nc.sync.dma_start(out=yT[et * 128:(et + 1) * 128, lo:lo + 512],
                              in_=ysb[:])

        # Q projection for chunk 0 runs up front.
        for t in range(QT):
            qproj(0, t)

        # ================= phase 2: attention, software-pipelined =================
        def normalize(avpA, avpB, avT_t, p, qc):
            """avT = av[0:64] * recip(av[64]). The two heads' denominator rows
            are replicated into one [128,512] PSUM tile via col-tiled K=1 MMs
            (concurrent), then a single DVE reciprocal covers both heads —
            reciprocal is an iterative-divide op (~6 cyc/elem), so halving the
            instruction count and keeping the PE-side MM dependent only on the
            cheap den copies keeps it off the critical path."""
            for g, avp, lrptag in ((0, avpA, "y"), (1, avpB, "misc")):
                den = npool.tile([1, 512], mybir.dt.float32r, tag="den",
                                 name=f"den{qc}_{p}_{g}")
                nc.vector.tensor_copy(out=den[:], in_=avp[64:65, :])
                lrp = ps.tile([128, 512], F32, tag=lrptag, name=f"lrp{qc}_{p}_{g}")
                nc.tensor.matmul(lrp[0:64, :], ones_row[:], den[:],
                                 start=True, stop=True)
                rinv = npool.tile([64, 512], F32, tag="rinv",
                                  name=f"rinv{qc}_{p}_{g}")
                nc.vector.reciprocal_approx_fast(out=rinv[:], in_=lrp[0:64, :])
                nc.vector.tensor_tensor(
                    out=avT_t[g * 64:g * 64 + 64, p, :], in0=avp[0:64, :],
                    in1=rinv[:], op=MUL)

        avT_tiles = {}
        pending_norm = deque()  # deferred normalizes, popped after next S/exp
        for qc in range(SC):
            lo = qc * 512
            fillers = deque()
            if qc + 1 < SC:
                for t in range(QT):
                    fillers.append(lambda t=t, s=qc + 1: qproj(s, t))
            if qc - 1 >= 0:
                prev_avT = avT_tiles[qc - 1]
                for et in range(ET):
                    fillers.append(lambda et=et, a=prev_avT, s=qc - 1:
                                   outproj_et(s, et, a))

            avT_t = avtpool.tile([128, QT, 512], BF16, tag="avT", name=f"avT{qc}")
            avT_tiles[qc] = avT_t
            for p in range(QT):
                # scores/exp run one kt ahead of AV; pair (p-1)'s normalize is
                # spliced in after this pair's first exp so its PE/DVE ops hide
                # under the exp stream instead of stalling the pair boundary.
                avpA = avpB = None
                pe_tiles = {}
                for kt in range(KT):
                    sT = ps.tile([128, 1024], F32, tag="s", bufs=2,
                                 name=f"s{qc}_{p}_{kt}")
                    nc.tensor.matmul(
                        sT[:, 0:512],
                        kT_sb[0:64, kt * 128:(kt + 1) * 128],
                        qT_sb[0:64, p, lo:lo + 512], start=True, stop=True)
                    nc.tensor.matmul(
                        sT[:, 512:1024],
                        kT_sb[64:128, kt * 128:(kt + 1) * 128],
                        qT_sb[64:128, p, lo:lo + 512], start=True, stop=True)
                    pe_t = pepool.tile([128, 1024], BF16, tag="pe",
                                       name=f"pe{qc}_{p}_{kt}")
                    if kt in GPS_KTS:
                        nc.gpsimd.tensor_scalar(
                            out=pe_t.bitcast(mybir.dt.int16), in0=sT[:],
                            scalar1=SCHR_A, scalar2=SCHR_B, op0=MUL, op1=ADD)
                    elif kt in DVE_KTS:
                        nc.vector.tensor_scalar(
                            out=pe_t.bitcast(mybir.dt.int16), in0=sT[:],
                            scalar1=SCHR_A, scalar2=SCHR_B, op0=MUL, op1=ADD)
                    else:
                        nc.scalar.activation(pe_t[:], sT[:], EXP,
                                             scale=float(SCALE))
                    pe_tiles[kt] = pe_t
                    if kt == 1:
                        while pending_norm:
                            pending_norm.popleft()()
                        # allocate accumulators after the deferred normalize of
                        # the previous pair has issued its reads (bufs=1 slots)
                        avpA = ps.tile([128, 512], F32, tag="avA",
                                       name=f"avpA{qc}_{p}")
                        avpB = ps.tile([128, 512], F32, tag="avB",
                                       name=f"avpB{qc}_{p}")
                    if kt >= 1:
                        pkt = kt - 1
                        pp = pe_tiles.pop(pkt)
                        nc.tensor.matmul(
                            avpA[0:65, :], vaug[:, pkt, :], pp[:, 0:512],
                            start=(pkt == 0), stop=False)
                        nc.tensor.matmul(
                            avpB[0:65, :], vaug[:, KT + pkt, :], pp[:, 512:1024],
                            start=(pkt == 0), stop=False)
                    # pop late so the y-tag rotation never stalls on the
                    # normalize reciprocal of the previous pair.
                    if kt in (7, 9, 11, 13, 15) and fillers:
                        fillers.popleft()()
                pp = pe_tiles.pop(KT - 1)
                nc.tensor.matmul(avpA[0:65, :], vaug[:, KT - 1, :], pp[:, 0:512],
                                 start=False, stop=True)
                nc.tensor.matmul(avpB[0:65, :], vaug[:, 2 * KT - 1, :],
                                 pp[:, 512:1024], start=False, stop=True)
                pending_norm.append(
                    lambda a=avpA, b=avpB, t=avT_t, p=p, q=qc:
                    normalize(a, b, t, p, q))
            while fillers:
                fillers.popleft()()
        while pending_norm:
            pending_norm.popleft()()
        # out-projection for the last chunk
        for et in range(ET):
            outproj_et(SC - 1, et, avT_tiles[SC - 1])
    nc.compile()
    return nc


def _shard_inputs(x, Wq, bq, Wk, bk, Wv, bv, Wo, bo):
    """Build the 8 per-core input maps (bf16 weights/activations)."""
    x = np.asarray(x, dtype=np.float32)
    in_maps = []
    for c in range(8):
        b, H = c // 2, c % 2
        heads = [8 * H + t for t in range(4)] + [8 * H + t + 4 for t in range(4)]
        # qT tile t holds (local head t -> partitions 0-63, local head t+4 -> 64-127)
        order = []
        for t in range(4):
            order.extend(range(heads[t] * 64, heads[t] * 64 + 64))
            order.extend(range(heads[t + 4] * 64, heads[t + 4] * 64 + 64))
        order = np.asarray(order)
        wq_p = np.ascontiguousarray(np.asarray(Wq, np.float32)[:, order]).astype(NPBF16)
        bq_p = np.ascontiguousarray(
            np.asarray(bq, np.float32)[order].reshape(4, 128).T)
        wo_p = np.ascontiguousarray(np.asarray(Wo, np.float32)[order, :]).astype(NPBF16)
        wk_s = np.ascontiguousarray(
            np.asarray(Wk, np.float32)[:, H * 128:(H + 1) * 128]).astype(NPBF16)
        wv_s = np.ascontiguousarray(
            np.asarray(Wv, np.float32)[:, H * 128:(H + 1) * 128]).astype(NPBF16)
        bk_s = np.ascontiguousarray(np.asarray(bk, np.float32)[H * 128:(H + 1) * 128]
                                    .reshape(128, 1))
        bv_s = np.ascontiguousarray(np.asarray(bv, np.float32)[H * 128:(H + 1) * 128]
                                    .reshape(128, 1))
        xT_b = np.ascontiguousarray(x[b].T.astype(NPBF16))
        in_maps.append({
            "xT": xT_b, "wq": wq_p, "wk": wk_s, "wv": wv_s, "wo": wo_p,
            "bqd": bq_p, "bkd": bk_s, "bvd": bv_s,
            "identd": np.eye(128, dtype=NPBF16),
            "onesd": np.ones((128, 64), dtype=NPBF16),
            "onesf": np.ones((1, 64), dtype=np.float32),
        })
    return in_maps


def kernel(x, Wq, bq, Wk, bk, Wv, bv, Wo, bo, _trace=False):
    if "nc" not in _NC_CACHE:
        _NC_CACHE["nc"] = build_nc()
    nc = _NC_CACHE["nc"]
    in_maps = _shard_inputs(x, Wq, bq, Wk, bk, Wv, bv, Wo, bo)
    res = run_bass_kernel_spmd(nc, in_maps, list(range(8)), trace=_trace)
    bo = np.asarray(bo, dtype=np.float32)
    out = np.empty((B, S, E), dtype=np.float32)
    for b in range(B):
        yT = res.results[2 * b]["yT"] + res.results[2 * b + 1]["yT"]
        out[b] = yT.T + bo
    if _trace:
        return out, res
    return out


# revision 51
# speedup vs baseline: 2.1009x; 1.0248x over previous
"""GQA self-attention kernel for Trainium2, sharded over 8 NeuronCores.

Problem: x[4, 2048, 1024], 16 heads / 4 KV groups / head_dim 64.
Sharding: batch (4) x head-half (2 KV groups each) = 8 cores.

v2 dataflow (all-bf16 PE path, transposed world: features on partitions):
  xT[1024,2048]b16 -> qT[512,2048], kT[128,2048], vT[128,2048]  (PE, bf16)
  vT --PE transpose--> vaug[seq,65] tiles (ones col appended -> softmax sums)
  scores s[k,q] = kT_g^T(d,kpos) . qT_h(d,q): K=64 contraction, the two KV
    groups run CONCURRENTLY in the upper/lower 64-row halves of the PE array
    (row tiling via base_partition-derived tile_position)
  exp: one ACT instruction per [128,2048] PSUM span (2 kt tiles x 2 heads)
    -> bf16 p tiles; amortizes the ~352-cycle ACT fixed cost
  av[65,q] += vaug^T p (row 64 = softmax denominator)
  avT_norm = av[0:64] * recip(av[64])  (DVE; denom row replicated via K=1 MM)
  yT[e,q] = Wo_p^T . avT_norm -> DRAM (f32)
The attention loop is software-pipelined; Q-projection (next chunk) and
out-projection (previous chunk) matmuls are spliced into the PE queue as
fillers during exp waits so the PE stays dense (HAM stays warm).
Host: y[b] = (yT[2b] + yT[2b+1]).T + bo
"""

import os
import sys
import numpy as np

# Coarse whole-tile dependency tracking: subtile AP-overlap analysis has
# proven racy for this kernel's 3D-sliced persistent tiles (intermittent
# first-run corruption); whole-tile deps are conservative and safe.
os.environ["BY_DEFAULT_DISABLE_SUBTILE_DEPS"] = "1"

sys.path.insert(0, "/opt/trn_rl_repo")

from collections import deque
from contextlib import ExitStack

import ml_dtypes

import concourse.bass as bass
import concourse.bacc as bacc
import concourse.mybir as mybir
from concourse import tile
from concourse.bass_utils import run_bass_kernel_spmd

F32 = mybir.dt.float32
BF16 = mybir.dt.bfloat16
NPBF16 = ml_dtypes.bfloat16

B, S, E = 4, 2048, 1024
NUM_HEADS, NUM_GROUPS, D = 16, 4, 64
CQ = 512          # q cols per core (8 heads)
CK = 128          # kv cols per core (2 groups)
ET = E // 128     # 8 embed K-tiles
SC = S // 512     # 4 seq chunks of 512
KT = S // 128     # 16 key tiles of 128
QT = CQ // 128    # 4 qT partition tiles (pair p: head p @0-63, head p+4 @64-127)
SCALE = 1.0 / np.sqrt(np.float32(D))
# Schraudolph fast-exp constants emitting bf16 bits as int16:
#   bf16_bits(e^(x*SCALE)) ~= int16((2^23/ln2)*SCALE/2^16 * x + (127*2^23-c)/2^16)
# c = 486411 minimizes rms relative error (~1.8%); used on a minority of
# attention kt-tiles (GPSIMD/DVE) to offload the ACT-bound exp stream.
SCHR_A = float((2.0 ** 23 / np.log(2.0)) * SCALE / 65536.0)
SCHR_B = float((127 * 2 ** 23 - 486411) / 65536.0)
GPS_KTS = ()              # GPSIMD cannot read PSUM (walrus birverifier)
DVE_KTS = (3, 7, 11, 15)  # exp tiles computed on DVE (25% offload)

_NC_CACHE = {}


def build_nc():
    nc = bacc.Bacc(None, target_bir_lowering=False)

    xT = nc.dram_tensor("xT", [E, S], BF16, kind="ExternalInput")
    wq = nc.dram_tensor("wq", [E, CQ], BF16, kind="ExternalInput")
    wk = nc.dram_tensor("wk", [E, CK], BF16, kind="ExternalInput")
    wv = nc.dram_tensor("wv", [E, CK], BF16, kind="ExternalInput")
    wo = nc.dram_tensor("wo", [CQ, E], BF16, kind="ExternalInput")
    bqd = nc.dram_tensor("bqd", [128, QT], F32, kind="ExternalInput")
    bkd = nc.dram_tensor("bkd", [128, 1], F32, kind="ExternalInput")
    bvd = nc.dram_tensor("bvd", [128, 1], F32, kind="ExternalInput")
    identd = nc.dram_tensor("identd", [128, 128], BF16, kind="ExternalInput")
    onesd = nc.dram_tensor("onesd", [128, 64], BF16, kind="ExternalInput")
    onesf = nc.dram_tensor("onesf", [1, 64], mybir.dt.float32r, kind="ExternalInput")
    yT = nc.dram_tensor("yT", [E, S], F32, kind="ExternalOutput")

    ADD = mybir.AluOpType.add
    MUL = mybir.AluOpType.mult
    EXP = mybir.ActivationFunctionType.Exp

    with tile.TileContext(nc) as tc, ExitStack() as ctx, \
            nc.allow_low_precision(reason="bf16 matmuls within 2e-2 tolerance"):
        const = ctx.enter_context(tc.tile_pool(name="const", bufs=1))
        wpool = ctx.enter_context(tc.tile_pool(name="wpool", bufs=1))
        big = ctx.enter_context(tc.tile_pool(name="big", bufs=1))
        pepool = ctx.enter_context(tc.tile_pool(name="pepool", bufs=4))
        avtpool = ctx.enter_context(tc.tile_pool(name="avtpool", bufs=2))
        npool = ctx.enter_context(tc.tile_pool(name="npool", bufs=2))
        ypool = ctx.enter_context(tc.tile_pool(name="ypool", bufs=2))
        # PSUM budget (16KB/partition = 8 banks, exact fit):
        #   s    [128,2048] f32  4 banks  (scores: 2 kt x 2 heads per j)
        #   avA  [128, 512] f32  1 bank   (head A attention accumulator)
        #   avB  [128, 512] f32  1 bank
        #   y    [128, 512] f32  1 bank   (out-proj + phase1 rotation)
        #   misc [128, 512] f32  1 bank   (q-proj fillers, lrp, transposes)
        ps = ctx.enter_context(tc.tile_pool(name="ps", bufs=1, space="PSUM"))

        # ---- constants ----
        ident = const.tile([128, 128], BF16)
        nc.sync.dma_start(out=ident[:], in_=identd[:, :])
        ones_row = const.tile([1, 64], mybir.dt.float32r)
        nc.sync.dma_start(out=ones_row[:], in_=onesf[0:1, :])

        # ---- weights + biases (kv + x chunk0 first: needed immediately) ----
        wk_sb = wpool.tile([128, ET, CK], BF16)
        wv_sb = wpool.tile([128, ET, CK], BF16)
        for et in range(ET):
            nc.sync.dma_start(out=wk_sb[:, et, :], in_=wk[et * 128:(et + 1) * 128, :])
            nc.sync.dma_start(out=wv_sb[:, et, :], in_=wv[et * 128:(et + 1) * 128, :])
        bk_sb = wpool.tile([128, 1], F32)
        nc.sync.dma_start(out=bk_sb[:], in_=bkd[:, :])
        bv_sb = wpool.tile([128, 1], F32)
        nc.sync.dma_start(out=bv_sb[:], in_=bvd[:, :])

        xT_sb = big.tile([128, ET, S], BF16)      # 32KB/partition
        # per-chunk qT tiles: with whole-tile dependency tracking, a single
        # big qT tensor would make every next-chunk q-projection filler write
        # falsely serialize against the in-flight score reads of the current
        # chunk. Separate tiles keep the filler stream independent.
        qT_ch = {sc: big.tile([128, QT, 512], BF16, tag=f"qT{sc}",
                              name=f"qT{sc}") for sc in range(SC)}
        kT_sb = big.tile([128, S], BF16)          # 4KB
        vT_sb = big.tile([128, S], BF16)          # 4KB
        vaug = big.tile([128, 2 * KT, 65], BF16)  # v natural + ones col

        for et in range(ET):
            nc.sync.dma_start(out=xT_sb[:, et, 0:512],
                              in_=xT[et * 128:(et + 1) * 128, 0:512])
        wq_sb = wpool.tile([128, ET, CQ], BF16)
        for et in range(ET):
            nc.sync.dma_start(out=wq_sb[:, et, :], in_=wq[et * 128:(et + 1) * 128, :])
        bq_sb = wpool.tile([128, QT], F32)
        nc.sync.dma_start(out=bq_sb[:], in_=bqd[:, :])
        for g in range(2):
            for kt in range(KT):
                nc.sync.dma_start(out=vaug[:, g * KT + kt, 64:65],
                                  in_=onesd[:, 0:1])
        for sc in range(1, SC):
            lo = sc * 512
            for et in range(ET):
                nc.sync.dma_start(
                    out=xT_sb[:, et, lo:lo + 512],
                    in_=xT[et * 128:(et + 1) * 128, lo:lo + 512])
        wo_sb = wpool.tile([128, QT, E], BF16)
        for t in range(QT):
            nc.sync.dma_start(out=wo_sb[:, t, :], in_=wo[t * 128:(t + 1) * 128, :])

        # ================= phase 1: K/V projections + transposes =================
        def kv_proj(sc):
            lo = sc * 512
            pk = ps.tile([128, 512], F32, tag="y", name=f"pk{sc}")
            for et in range(ET):
                nc.tensor.matmul(pk[:], wk_sb[:, et, :], xT_sb[:, et, lo:lo + 512],
                                 start=(et == 0), stop=(et == ET - 1))
            nc.vector.tensor_scalar(out=kT_sb[:, lo:lo + 512], in0=pk[:],
                                    scalar1=bk_sb[:, 0:1], scalar2=None, op0=ADD)
            pv = ps.tile([128, 512], F32, tag="misc", name=f"pv{sc}")
            for et in range(ET):
                nc.tensor.matmul(pv[:], wv_sb[:, et, :], xT_sb[:, et, lo:lo + 512],
                                 start=(et == 0), stop=(et == ET - 1))
            nc.vector.tensor_scalar(out=vT_sb[:, lo:lo + 512], in0=pv[:],
                                    scalar1=bv_sb[:, 0:1], scalar2=None, op0=ADD)
            for ktl in range(4):
                kt = sc * 4 + ktl
                ptr = ps.tile([128, 128], BF16, tag="y", name=f"ptr{kt}")
                nc.tensor.transpose(ptr[:], vT_sb[:, kt * 128:(kt + 1) * 128], ident[:])
                for g in range(2):
                    nc.vector.tensor_copy(
                        out=vaug[:, g * KT + kt, 0:64], in_=ptr[:, g * 64:(g + 1) * 64])

        for sc in range(SC):
            kv_proj(sc)

        # ---- helpers issued inline or as fillers ----
        def qproj(sc, t):
            """One qT tile: 8-matmul accumulation + bias, issued atomically
            (tag-rotation safety: nothing else may allocate this tag between
            a tile's first write and its last read)."""
            lo = sc * 512
            pq = ps.tile([128, 512], F32, tag="misc", name=f"pq{sc}_{t}")
            for et in range(ET):
                nc.tensor.matmul(pq[:], wq_sb[:, et, t * 128:(t + 1) * 128],
                                 xT_sb[:, et, lo:lo + 512],
                                 start=(et == 0), stop=(et == ET - 1))
            nc.vector.tensor_scalar(
                out=qT_ch[sc][:, t, :], in0=pq[:],
                scalar1=bq_sb[:, t:t + 1], scalar2=None, op0=ADD)

        def outproj_et(qc, et, avT_t):
            lo = qc * 512
            yp = ps.tile([128, 512], F32, tag="y", name=f"yp{qc}_{et}")
            for t in range(QT):
                nc.tensor.matmul(yp[:], wo_sb[:, t, et * 128:(et + 1) * 128],
                                 avT_t[:, t, :], start=(t == 0), stop=(t == QT - 1))
            ysb = ypool.tile([128, 512], F32, tag="ysb", name=f"ysb{qc}_{et}")
            nc.vector.tensor_copy(out=ysb[:], in_=yp[:])
            nc.sync.dma_start(out=yT[et * 128:(et + 1) * 128, lo:lo + 512],
                              in_=ysb[:])

        # Q projection for chunk 0 runs up front.
        for t in range(QT):
            qproj(0, t)

        # ================= phase 2: attention, software-pipelined =================
        def normalize(avpA, avpB, avT_t, p, qc):
            """avT = av[0:64] * recip(av[64]). The two heads' denominator rows
            are replicated into one [128,512] PSUM tile via col-tiled K=1 MMs
            (concurrent), then a single DVE reciprocal covers both heads —
            reciprocal is an iterative-divide op (~6 cyc/elem), so halving the
            instruction count and keeping the PE-side MM dependent only on the
            cheap den copies keeps it off the critical path."""
            for g, avp, lrptag in ((0, avpA, "y"), (1, avpB, "misc")):
                den = npool.tile([1, 512], mybir.dt.float32r, tag="den",
                                 name=f"den{qc}_{p}_{g}")
                nc.vector.tensor_copy(out=den[:], in_=avp[64:65, :])
                lrp = ps.tile([128, 512], F32, tag=lrptag, name=f"lrp{qc}_{p}_{g}")
                nc.tensor.matmul(lrp[0:64, :], ones_row[:], den[:],
                                 start=True, stop=True)
                rinv = npool.tile([64, 512], F32, tag="rinv",
                                  name=f"rinv{qc}_{p}_{g}")
                nc.vector.reciprocal_approx_fast(out=rinv[:], in_=lrp[0:64, :])
                nc.vector.tensor_tensor(
                    out=avT_t[g * 64:g * 64 + 64, p, :], in0=avp[0:64, :],
                    in1=rinv[:], op=MUL)

        avT_tiles = {}
        pending_norm = deque()  # deferred normalizes, popped after next S/exp
        for qc in range(SC):
            lo = qc * 512
            fillers = deque()
            if qc + 1 < SC:
                for t in range(QT):
                    fillers.append(lambda t=t, s=qc + 1: qproj(s, t))
            if qc - 1 >= 0:
                prev_avT = avT_tiles[qc - 1]
                for et in range(ET):
                    fillers.append(lambda et=et, a=prev_avT, s=qc - 1:
                                   outproj_et(s, et, a))

            avT_t = avtpool.tile([128, QT, 512], BF16, tag="avT", name=f"avT{qc}")
            avT_tiles[qc] = avT_t
            for p in range(QT):
                # scores/exp run one kt ahead of AV; pair (p-1)'s normalize is
                # spliced in after this pair's first exp so its PE/DVE ops hide
                # under the exp stream instead of stalling the pair boundary.
                avpA = avpB = None
                pe_tiles = {}
                for kt in range(KT):
                    sT = ps.tile([128, 1024], F32, tag="s", bufs=2,
                                 name=f"s{qc}_{p}_{kt}")
                    nc.tensor.matmul(
                        sT[:, 0:512],
                        kT_sb[0:64, kt * 128:(kt + 1) * 128],
                        qT_ch[qc][0:64, p, :], start=True, stop=True)
                    nc.tensor.matmul(
                        sT[:, 512:1024],
                        kT_sb[64:128, kt * 128:(kt + 1) * 128],
                        qT_ch[qc][64:128, p, :], start=True, stop=True)
                    pe_t = pepool.tile([128, 1024], BF16, tag="pe",
                                       name=f"pe{qc}_{p}_{kt}")
                    if kt in GPS_KTS:
                        nc.gpsimd.tensor_scalar(
                            out=pe_t.bitcast(mybir.dt.int16), in0=sT[:],
                            scalar1=SCHR_A, scalar2=SCHR_B, op0=MUL, op1=ADD)
                    elif kt in DVE_KTS:
                        nc.vector.tensor_scalar(
                            out=pe_t.bitcast(mybir.dt.int16), in0=sT[:],
                            scalar1=SCHR_A, scalar2=SCHR_B, op0=MUL, op1=ADD)
                    else:
                        nc.scalar.activation(pe_t[:], sT[:], EXP,
                                             scale=float(SCALE))
                    pe_tiles[kt] = pe_t
                    if kt == 1:
                        while pending_norm:
                            pending_norm.popleft()()
                        # allocate accumulators after the deferred normalize of
                        # the previous pair has issued its reads (bufs=1 slots)
                        avpA = ps.tile([128, 512], F32, tag="avA",
                                       name=f"avpA{qc}_{p}")
                        avpB = ps.tile([128, 512], F32, tag="avB",
                                       name=f"avpB{qc}_{p}")
                    if kt >= 1:
                        pkt = kt - 1
                        pp = pe_tiles.pop(pkt)
                        nc.tensor.matmul(
                            avpA[0:65, :], vaug[:, pkt, :], pp[:, 0:512],
                            start=(pkt == 0), stop=False)
                        nc.tensor.matmul(
                            avpB[0:65, :], vaug[:, KT + pkt, :], pp[:, 512:1024],
                            start=(pkt == 0), stop=False)
                    # pop late so the y-tag rotation never stalls on the
                    # normalize reciprocal of the previous pair.
                    if kt in (7, 9, 11, 13, 15) and fillers:
                        fillers.popleft()()
                pp = pe_tiles.pop(KT - 1)
                nc.tensor.matmul(avpA[0:65, :], vaug[:, KT - 1, :], pp[:, 0:512],
                                 start=False, stop=True)
                nc.tensor.matmul(avpB[0:65, :], vaug[:, 2 * KT - 1, :],
                                 pp[:, 512:1024], start=False, stop=True)
                pending_norm.append(
                    lambda a=avpA, b=avpB, t=avT_t, p=p, q=qc:
                    normalize(a, b, t, p, q))
            while fillers:
                fillers.popleft()()
        while pending_norm:
            pending_norm.popleft()()
        # out-projection for the last chunk
        for et in range(ET):
            outproj_et(SC - 1, et, avT_tiles[SC - 1])
    nc.compile()
    return nc


def _shard_inputs(x, Wq, bq, Wk, bk, Wv, bv, Wo, bo):
    """Build the 8 per-core input maps (bf16 weights/activations)."""
    x = np.asarray(x, dtype=np.float32)
    in_maps = []
    for c in range(8):
        b, H = c // 2, c % 2
        heads = [8 * H + t for t in range(4)] + [8 * H + t + 4 for t in range(4)]
        # qT tile t holds (local head t -> partitions 0-63, local head t+4 -> 64-127)
        order = []
        for t in range(4):
            order.extend(range(heads[t] * 64, heads[t] * 64 + 64))
            order.extend(range(heads[t + 4] * 64, heads[t + 4] * 64 + 64))
        order = np.asarray(order)
        wq_p = np.ascontiguousarray(np.asarray(Wq, np.float32)[:, order]).astype(NPBF16)
        bq_p = np.ascontiguousarray(
            np.asarray(bq, np.float32)[order].reshape(4, 128).T)
        wo_p = np.ascontiguousarray(np.asarray(Wo, np.float32)[order, :]).astype(NPBF16)
        wk_s = np.ascontiguousarray(
            np.asarray(Wk, np.float32)[:, H * 128:(H + 1) * 128]).astype(NPBF16)
        wv_s = np.ascontiguousarray(
            np.asarray(Wv, np.float32)[:, H * 128:(H + 1) * 128]).astype(NPBF16)
        bk_s = np.ascontiguousarray(np.asarray(bk, np.float32)[H * 128:(H + 1) * 128]
                                    .reshape(128, 1))
        bv_s = np.ascontiguousarray(np.asarray(bv, np.float32)[H * 128:(H + 1) * 128]
                                    .reshape(128, 1))
        xT_b = np.ascontiguousarray(x[b].T.astype(NPBF16))
        in_maps.append({
            "xT": xT_b, "wq": wq_p, "wk": wk_s, "wv": wv_s, "wo": wo_p,
            "bqd": bq_p, "bkd": bk_s, "bvd": bv_s,
            "identd": np.eye(128, dtype=NPBF16),
            "onesd": np.ones((128, 64), dtype=NPBF16),
            "onesf": np.ones((1, 64), dtype=np.float32),
        })
    return in_maps


def kernel(x, Wq, bq, Wk, bk, Wv, bv, Wo, bo, _trace=False):
    if "nc" not in _NC_CACHE:
        _NC_CACHE["nc"] = build_nc()
    nc = _NC_CACHE["nc"]
    in_maps = _shard_inputs(x, Wq, bq, Wk, bk, Wv, bv, Wo, bo)
    res = run_bass_kernel_spmd(nc, in_maps, list(range(8)), trace=_trace)
    bo = np.asarray(bo, dtype=np.float32)
    out = np.empty((B, S, E), dtype=np.float32)
    for b in range(B):
        yT = res.results[2 * b]["yT"] + res.results[2 * b + 1]["yT"]
        out[b] = yT.T + bo
    if _trace:
        return out, res
    return out
